# revision 1
# baseline (speedup 1.0000x reference)
"""Trainium2 Bass kernel for nn_MultiHeadAttention_52192442581071.

Reference math:
    qp  = query @ Wq.T                                   [bs, D]
    kp  = keys @ Wk.T ; vp = values @ Wv.T               [sl, bs, D]
    dot = (qp * kp).reshape(sl,bs,H,64).sum(-1)/8        [sl, bs, H]
    w   = log_softmax(dot, axis=0)
    attn= (w[...,None] * vp.reshape(sl,bs,H,64)).sum(0)  [bs, H, 64]
    out = attn.reshape(bs,-1) @ Wo.T                     [bs, D]

Algebraic restructuring (exact in exact arithmetic):
    r[b,h,:]   = sum_{j in head h} qp[b,j] * Wk[j,:] / 8          (small)
    dot[s,b,h] = keys[s,b,:] . r[b,h,:]                            (matmul)
    z[b,h,:]   = sum_s (dot-lse) * values[s,b,:] = P - lse*V
        P[b,h,:] = sum_s dot[s,b,h] * values[s,b,:]                (matmul)
        V[b,:]   = sum_s values[s,b,:]          (ones-column trick)
    attn[b,h,n] = P[b,h,:] . Wv[h*64+n,:] - lse[b,h]*(V[b,:] . Wv[h*64+n,:])
    out = attn @ Wo.T                                              (small)

Performance notes:
  * All wire data is bf16 (fp32 PSUM accumulation, fp32 softmax stats):
    halves the HBM traffic, which is the binding resource.
  * Host-side sharding pre-packs layouts (keys transposed to [d, s],
    weights pre-permuted), removing all on-chip weight/key transposes.
  * dot is stored as (dot - 4) so the bf16 dotT tiles carry the
    near-zero top logits at full precision; the shift is absorbed by
    the log-sum-exp correction.
  * The -lse*V correction is applied in attn space (transpose and the
    Wv projection are linear), so the P transposes and projections do
    not wait for the softmax statistics.
  * All bulk DMAs ride the single SP queue (measured fastest on HW;
    multi-queue DMA loses ~30% bandwidth on real silicon), the output
    DMA rides the Pool queue so the next invocation's stream is not
    head-of-line blocked, and loop-invariant constants load once.

Sharding: data-parallel over bs. Core i handles batch rows [4i, 4i+4).
No collectives; same program on all 8 cores with different inputs.
"""

import sys

if "/opt/trn_rl_repo" not in sys.path:
    sys.path.insert(0, "/opt/trn_rl_repo")

import numpy as np

import concourse.bass as bass
import concourse.mybir as mybir
import concourse.tile as tile
from concourse import bacc, bass_utils

# Problem constants (hardcoded per contract)
H = 16          # num heads
NHID = 64
D = 1024
SL = 2048
BS = 32
NCORES = 8
B = BS // NCORES  # 4 local batch elements per core

FP32 = mybir.dt.float32
BF16 = mybir.dt.bfloat16
NP_BF16 = mybir.dt.np(BF16)
Exp = mybir.ActivationFunctionType.Exp
Ln = mybir.ActivationFunctionType.Ln
X = mybir.AxisListType.X

import os
KP_BUFS = int(os.environ.get("KP_BUFS", "4"))
VP_BUFS = int(os.environ.get("VP_BUFS", "4"))

DC = D // 128     # 8 d-chunks
JC = D // 128     # 8 j-chunks
NSB = int(os.environ.get("NSB", "4"))  # kv slabs per sequence
SBLK = SL // NSB   # s per slab
NST = SBLK // 128  # s-tiles per slab
NBLK = SBLK // 512  # 512-wide dot blocks per slab
LOGIT_SHIFT = -4.0  # dot stored as dot-4: top logits near 0 in bf16


def build_program(loop_n=1, loop_bodies=1):
    nc = bacc.Bacc(
        "TRN2", target_bir_lowering=False, debug=False,
        enable_asserts=False, num_devices=1,
    )
    qT_d = nc.dram_tensor("qT", [128, DC, B], BF16, kind="ExternalInput").ap()
    kT_d = nc.dram_tensor("kT", [B, NSB, 128, DC, SBLK], BF16,
                          kind="ExternalInput").ap()
    v_d = nc.dram_tensor("vv", [B, NSB, 128, NST, D], BF16,
                         kind="ExternalInput").ap()
    wqT_d = nc.dram_tensor("wqT", [128, DC, D], BF16, kind="ExternalInput").ap()
    wk_d = nc.dram_tensor("wk", [128, JC, D], BF16, kind="ExternalInput").ap()
    wvT_d = nc.dram_tensor("wvT", [128, DC, D], BF16, kind="ExternalInput").ap()
    woT_d = nc.dram_tensor("woT", [128, JC, D], BF16, kind="ExternalInput").ap()
    out_d = nc.dram_tensor("out", [B, D], FP32, kind="ExternalOutput").ap()
    ident_d = nc.inline_tensor(np.eye(33, dtype=np.float32), "ident").ap()
    # mask[p, jc, h] = 1/8 if head(jc*128+p) == h else 0
    mask_np = np.zeros((128, JC, H), dtype=np.float32)
    for jc in range(JC):
        for p in range(128):
            mask_np[p, jc, (jc * 128 + p) // NHID] = 0.125
    mask_d = nc.inline_tensor(mask_np, "headmask").ap()
    # maskT8[h, jc, p] = 1 if head(jc*128+p) == h else 0 (for lse scatter)
    maskT_np = np.zeros((H, JC, 128), dtype=np.float32)
    for jc in range(JC):
        for p in range(128):
            maskT_np[(jc * 128 + p) // NHID, jc, p] = 1.0
    maskT_d = nc.inline_tensor(maskT_np, "headmaskT").ap()

    from contextlib import ExitStack
    with tile.TileContext(nc) as tc:
        with ExitStack() as ctx:
            # pools hoisted out of the loop: no per-iteration alloc/drain
            pools = dict(
                const=ctx.enter_context(tc.tile_pool(name="const", bufs=1)),
                kp=ctx.enter_context(tc.tile_pool(name="kp", bufs=KP_BUFS)),
                vp=ctx.enter_context(tc.tile_pool(name="vp", bufs=VP_BUFS)),
                d17p=ctx.enter_context(tc.tile_pool(name="d17p", bufs=6)),
                zp=ctx.enter_context(tc.tile_pool(name="zp", bufs=4)),
                stats=ctx.enter_context(tc.tile_pool(name="stats", bufs=1)),
                pre=ctx.enter_context(tc.tile_pool(name="pre", bufs=1)),
                psum_tr=ctx.enter_context(
                    tc.tile_pool(name="psum_tr", bufs=2, space="PSUM")),
                psum_acc=ctx.enter_context(
                    tc.tile_pool(name="psum_acc", bufs=2, space="PSUM")),
                psum_pv=ctx.enter_context(
                    tc.tile_pool(name="psum_pv", bufs=2, space="PSUM")),
            )
            # loop-invariant constants: loaded once, before the loop, so
            # iteration i+1's DMA queue is not blocked behind iteration
            # i's late readers of these tiles.
            const = pools["const"]
            ident = const.tile([33, 33], FP32, name="ident_sb")
            nc.sync.dma_start(ident[:], ident_d)
            mask_sb = const.tile([128, JC, H], FP32, name="mask_sb")
            nc.sync.dma_start(mask_sb[:], mask_d)
            maskT = const.tile([H, JC, 128], FP32, name="maskT_sb")
            nc.sync.dma_start(maskT[:], maskT_d)
            # dot rows 16..31 zero, row 32 ones; rows 0..15 are the only
            # ones rewritten inside the loop.
            dot_tiles = []
            for i in range(2):
                dot_b = const.tile([33, SL], FP32, name=f"dot{i}")
                nc.vector.memset(dot_b[:], 0.0)
                nc.vector.memset(dot_b[32:33, :], 1.0)
                dot_tiles.append(dot_b)
            consts = (ident, mask_sb, maskT, dot_tiles)
            if loop_n > 1:
                with tc.For_i(0, loop_n, 1,
                              staggered_reset=os.environ.get("SRESET", "0") == "1"):
                    for _ in range(loop_bodies):
                        _body(tc, pools, consts, out_d, qT_d, kT_d, v_d,
                              wqT_d, wk_d, wvT_d, woT_d)
            else:
                _body(tc, pools, consts, out_d, qT_d, kT_d, v_d, wqT_d, wk_d,
                      wvT_d, woT_d)
    nc.compile()
    return nc


def _body(tc, pools, consts, out_d, qT_d, kT_d, v_d, wqT_d, wk_d, wvT_d,
          woT_d):
    nc = tc.nc
    ident, mask_sb, maskT, dot_tiles = consts
    if True:
        const = pools["const"]
        kp = pools["kp"]
        vp = pools["vp"]
        d17p = pools["d17p"]
        zp = pools["zp"]
        stats = pools["stats"]
        pre = pools["pre"]
        psum_tr = pools["psum_tr"]
        psum_acc = pools["psum_acc"]
        psum_pv = pools["psum_pv"]

        # ---- weight/const DMAs ------------------------------------------
        # DMAQ: which queues carry DMAs. Real-HW probe showed a single
        # queue reaches full bandwidth while SWDGE (Pool) DMAs are slow.
        dmaq_mode = os.environ.get("DMAQ", "one")
        qs = {"one": [nc.sync],
              "wact": [nc.sync],
              "two": [nc.sync, nc.scalar],
              "three": [nc.sync, nc.scalar, nc.gpsimd]}[dmaq_mode]
        nq = len(qs)
        # weight queue: in "wact" mode the 8 MiB of weights ride the Act
        # queue, freeing the SP queue for the kv stream.
        wq_eng = nc.scalar if dmaq_mode == "wact" else qs[0]
        wqT = pre.tile([128, DC, D], BF16, name="wqT_sb")
        wk = pre.tile([128, JC, D], BF16, name="wk_sb")
        if nq == 1:
            wq_eng.dma_start(wqT[:], wqT_d)
            wq_eng.dma_start(wk[:], wk_d)
        else:
            qs[0].dma_start(wqT[:, 0:4, :], wqT_d[:, 0:4, :])
            qs[nq - 1].dma_start(wqT[:, 4:8, :], wqT_d[:, 4:8, :])
            qs[(nq - 1) // 2].dma_start(wk[:, 0:4, :], wk_d[:, 0:4, :])
            qs[0].dma_start(wk[:, 4:8, :], wk_d[:, 4:8, :])
        # (wvT/woT DMAs are issued mid-stream: they are only needed by
        # the tail projections.)
        wvT = const.tile([128, DC, D], BF16, name="wvT")
        woT = const.tile([128, JC, D], BF16, name="woT")

        qT = pre.tile([128, DC, B], BF16, name="qT_sb")
        nc.sync.dma_start(qT[:], qT_d)

        # rT[p, dc, b, h] = r[b, h, dc*128+p]  (r includes the 1/8 scale)
        rT = const.tile([128, DC, B, H], BF16, name="rT")
        # zT[p, dc, b, h] = P[b, h, dc*128+p]  (uncorrected attention sum)
        zT = const.tile([128, DC, B, H], BF16, name="zT")
        # VT[p, dc, b] = V[b, dc*128+p],  V[b] = sum_s values[s, b, :]
        VT = const.tile([128, DC, B], BF16, name="VT")
        nl128 = const.tile([128, JC, B], FP32, name="nl128")
        VWv = const.tile([128, JC, B], FP32, name="VWv")
        prod = const.tile([128, JC, B], FP32, name="prod")
        attnT = const.tile([128, JC, B], BF16, name="attnT")
        out_sb = const.tile([B, D], FP32, name="out_sb")

        # ---- preamble: qp, r --------------------------------------------
        # qpT[p, jc, b] = qp[b, jc*128+p]
        qpT = pre.tile([128, JC, B], FP32, name="qpT")
        for jc in range(JC):
            ps = psum_acc.tile([128, B], FP32, tag="acc", name=f"ps_qp{jc}")
            for dc in range(DC):
                nc.tensor.matmul(
                    ps[:], wqT[:, dc, jc * 128:(jc + 1) * 128], qT[:, dc, :],
                    start=(dc == 0), stop=(dc == DC - 1))
            nc.vector.tensor_copy(qpT[:, jc, :], ps[:])

        # Q[p, jc, b, h] = qp[b, jc*128+p]/8 if head(jc*128+p)==h else 0
        Q = pre.tile([128, JC, B, H], BF16, name="Q")
        nc.vector.tensor_tensor(
            Q[:],
            qpT[:, :, :, None].to_broadcast((128, JC, B, H)),
            mask_sb[:, :, None, :].to_broadcast((128, JC, B, H)),
            mybir.AluOpType.mult)

        # rT[d, (b,h)] = sum_j Wk[j, d] * Q[j, (b,h)]
        for dc in range(DC):
            ps = psum_acc.tile([128, B * H], FP32, tag="acc", name=f"ps_r{dc}")
            for jc in range(JC):
                nc.tensor.matmul(
                    ps[:], wk[:, jc, dc * 128:(dc + 1) * 128],
                    Q[:, jc, :, :], start=(jc == 0), stop=(jc == JC - 1))
            nc.vector.tensor_copy(rT[:, dc, :, :], ps[:])

        # ---- main loop: stream keys/values ------------------------------
        z_tiles = []
        S_all = stats.tile([16, B], FP32, tag="S", name="S_all")
        for b in range(B):
            dot_b = dot_tiles[b % 2]
            pv0 = psum_pv.tile([33, 512], FP32, tag="pv0", name=f"pv0_{b}")
            pv1 = psum_pv.tile([33, 512], FP32, tag="pv1", name=f"pv1_{b}")
            scratch = stats.tile([16, 512], FP32, tag="scratch", bufs=2,
                                 name=f"scr{b}")
            S_parts = stats.tile([16, 4], FP32, tag="Sp", bufs=2,
                                 name=f"Sp{b}")
            for sblk in range(NSB):
                # kv slabs round-robin over the DMA queues in use
                i = NSB * b + sblk
                kT = kp.tile([128, DC, SBLK], BF16, tag="kT",
                             name=f"kT_{b}_{sblk}")
                qs[i % nq].dma_start(kT[:], kT_d[b, sblk])
                v_t = vp.tile([128, NST, D], BF16, tag="v",
                              name=f"v_{b}_{sblk}")
                qs[(i + 1) % nq].dma_start(v_t[:], v_d[b, sblk])
                if sblk == 0 and b == 1:
                    (wq_eng if nq == 1 else qs[nq - 1]).dma_start(wvT[:], wvT_d)
                if sblk == 0 and b == 2:
                    (wq_eng if nq == 1 else qs[nq - 1]).dma_start(woT[:], woT_d)
                # dot[h, s'] (shifted by LOGIT_SHIFT) for this slab
                for blk in range(NBLK):
                    s0 = sblk * SBLK + blk * 512
                    ps_dot = psum_acc.tile([16, 512], FP32, tag="acc",
                                           name=f"ps_dot{b}_{sblk}_{blk}")
                    for dc in range(DC):
                        nc.tensor.matmul(
                            ps_dot[:], rT[:, dc, b, :],
                            kT[:, dc, blk * 512:(blk + 1) * 512],
                            start=(dc == 0), stop=(dc == DC - 1))
                    nc.vector.tensor_scalar_add(
                        dot_b[0:16, s0:s0 + 512], ps_dot[:], LOGIT_SHIFT)
                    # partial softmax denominator for this 512-block:
                    # dot_b is dot-4 (|dot| <~ 6) so exp cannot overflow
                    # and no running-max subtraction is needed.
                    nc.scalar.activation(
                        scratch[:, 0:512], dot_b[0:16, s0:s0 + 512], Exp,
                        bias=0.0, scale=1.0,
                        accum_out=S_parts[:, sblk * NBLK + blk:
                                          sblk * NBLK + blk + 1])
                # dotT tiles + P/V accumulation
                for st in range(NST):
                    cols = slice(sblk * SBLK + st * 128,
                                 sblk * SBLK + (st + 1) * 128)
                    ps_t = psum_tr.tile([128, 33], FP32, tag="tr",
                                        name=f"ps_dt{b}_{sblk}_{st}")
                    nc.tensor.transpose(ps_t[:], dot_b[:, cols], ident[:])
                    d17 = d17p.tile([128, 33], BF16, tag="d17",
                                    name=f"d17_{b}_{sblk}_{st}")
                    nc.vector.tensor_copy(d17[:], ps_t[:])
                    first = (sblk == 0 and st == 0)
                    last = (sblk == NSB - 1 and st == NST - 1)
                    nc.tensor.matmul(pv0[:], d17[:], v_t[:, st, 0:512],
                                     start=first, stop=last)
                    nc.tensor.matmul(pv1[:], d17[:], v_t[:, st, 512:1024],
                                     start=first, stop=last)
            # ---- per-b: finish softmax denominator ----------------------
            nc.vector.reduce_sum(S_all[:, b:b + 1], S_parts[:], axis=X)
            # ---- per-b: drain PSUM (P rows + V row), transpose P --------
            V_sb = stats.tile([33, D], FP32, tag="V", name=f"V{b}")
            nc.vector.tensor_copy(V_sb[32:33, 0:512], pv0[32:33, :])
            nc.vector.tensor_copy(V_sb[32:33, 512:1024], pv1[32:33, :])
            for dc in range(DC):
                ps_vt = psum_tr.tile([128, 1], FP32, tag="tr",
                                     name=f"ps_vt{b}_{dc}")
                nc.tensor.transpose(
                    ps_vt[:], V_sb[32:33, dc * 128:(dc + 1) * 128],
                    ident[32:33, 32:33])
                nc.vector.tensor_copy(VT[:, dc, b:b + 1], ps_vt[:])
            z_b = zp.tile([16, D], FP32, tag="z", name=f"z{b}")
            nc.vector.tensor_copy(z_b[:, 0:512], pv0[0:16, :])
            nc.vector.tensor_copy(z_b[:, 512:1024], pv1[0:16, :])
            z_tiles.append(z_b)
            for dc in range(DC):
                ps = psum_tr.tile([128, 16], FP32, tag="tr",
                                  name=f"ps_z{b}_{dc}")
                nc.tensor.transpose(
                    ps[:], z_b[:, dc * 128:(dc + 1) * 128], ident[:16, :16])
                nc.vector.tensor_copy(zT[:, dc, b, :], ps[:])

        # ---- tail ------------------------------------------------------
        # lse: one Ln pass; scatter -lse[b,h] to the attnT row layout.
        lnS = stats.tile([16, B], FP32, tag="lnS", name="lnS")
        nc.scalar.activation(lnS[:], S_all[:], Ln)
        neg_lse = stats.tile([16, B], FP32, tag="neg_lse", name="nlse")
        nc.vector.tensor_scalar_mul(neg_lse[:], lnS[:], -1.0)
        for jc in range(JC):
            ps_n = psum_tr.tile([128, B], FP32, tag="tr", name=f"ps_nl{jc}")
            nc.tensor.matmul(ps_n[:], maskT[:, jc, :], neg_lse[:],
                             start=True, stop=True)
            nc.vector.tensor_copy(nl128[:, jc, :], ps_n[:])
        # VWv[p, jc, b] = V[b, :] . Wv[jc*128+p, :]
        for jc in range(JC):
            ps_v = psum_acc.tile([128, B], FP32, tag="acc", name=f"ps_vw{jc}")
            for dc in range(DC):
                nc.tensor.matmul(
                    ps_v[:], wvT[:, dc, jc * 128:(jc + 1) * 128],
                    VT[:, dc, :], start=(dc == 0), stop=(dc == DC - 1))
            nc.vector.tensor_copy(VWv[:, jc, :], ps_v[:])
        # prod = (-lse) * VWv, then attn = P.Wv + prod fused into the
        # PSUM-drain copies of the P.Wv matmuls. The out-projection
        # accumulation is interleaved per jc so PE overlaps it with the
        # remaining attn groups instead of running two serial phases.
        nc.vector.tensor_tensor(prod[:], nl128[:], VWv[:],
                                mybir.AluOpType.mult)
        ps_o0 = psum_tr.tile([B, 512], FP32, tag="tr", name="ps_o0")
        ps_o1 = psum_tr.tile([B, 512], FP32, tag="tr", name="ps_o1")
        for jc in range(JC):
            for hlf in range(2):
                h = 2 * jc + hlf
                ps_a = psum_acc.tile([64, B], FP32, tag="acc",
                                     name=f"ps_a{h}")
                for dc in range(DC):
                    nc.tensor.matmul(
                        ps_a[:], wvT[:, dc, h * 64:(h + 1) * 64],
                        zT[:, dc, :, h], start=(dc == 0), stop=(dc == DC - 1))
                rows = slice(64 * hlf, 64 * hlf + 64)
                nc.vector.tensor_tensor(attnT[rows, jc, :], ps_a[:],
                                        prod[rows, jc, :],
                                        mybir.AluOpType.add)
            nc.tensor.matmul(ps_o0[:], attnT[:, jc, :], woT[:, jc, 0:512],
                             start=(jc == 0), stop=(jc == JC - 1))
            nc.tensor.matmul(ps_o1[:], attnT[:, jc, :], woT[:, jc, 512:1024],
                             start=(jc == 0), stop=(jc == JC - 1))
        nc.vector.tensor_copy(out_sb[:, 0:512], ps_o0[:])
        nc.gpsimd.dma_start(out_d[:, 0:512], out_sb[:, 0:512])
        nc.vector.tensor_copy(out_sb[:, 512:1024], ps_o1[:])
        nc.gpsimd.dma_start(out_d[:, 512:1024], out_sb[:, 512:1024])


_NC_CACHE = {}


def get_program():
    if "nc" not in _NC_CACHE:
        _NC_CACHE["nc"] = build_program()
    return _NC_CACHE["nc"]


def make_in_maps(query, keys, values, Wq, Wk, Wv, Wo):
    """Host-side shard + layout packing (pure permutation / dtype cast)."""
    query = np.asarray(query, dtype=np.float32).astype(NP_BF16)
    keys = np.asarray(keys, dtype=np.float32).astype(NP_BF16)
    values = np.asarray(values, dtype=np.float32).astype(NP_BF16)

    def permW(W):  # [p, dc, j] = W[j, dc*128+p]
        W = np.asarray(W, dtype=np.float32).astype(NP_BF16)
        return np.ascontiguousarray(
            W.T.reshape(DC, 128, D).transpose(1, 0, 2))

    def natW(W):  # [p, jc, d] = W[jc*128+p, d]
        W = np.asarray(W, dtype=np.float32).astype(NP_BF16)
        return np.ascontiguousarray(W.reshape(JC, 128, D).transpose(1, 0, 2))

    wqT = permW(Wq)
    wk = natW(Wk)
    wvT = permW(Wv)
    woT = permW(Wo)

    # kT[b, sblk, p, dc, s'] = keys[sblk*SBLK+s', b, dc*128+p]
    kT_all = keys.transpose(1, 2, 0).reshape(BS, DC, 128, NSB, SBLK)
    kT_all = kT_all.transpose(0, 3, 2, 1, 4)
    # vv[b, sblk, p, st, d] = values[sblk*SBLK+st*128+p, b, d]
    v_all = values.reshape(NSB, NST, 128, BS, D).transpose(3, 0, 2, 1, 4)

    in_maps = []
    for i in range(NCORES):
        sl = slice(B * i, B * (i + 1))
        qT = np.ascontiguousarray(
            query[sl].T.reshape(DC, 128, B).transpose(1, 0, 2))
        in_maps.append({
            "qT": qT,
            "kT": np.ascontiguousarray(kT_all[sl]),
            "vv": np.ascontiguousarray(v_all[sl]),
            "wqT": wqT, "wk": wk, "wvT": wvT, "woT": woT,
        })
    return in_maps


def kernel(query, keys, values, Wq, Wk, Wv, Wo):
    nc = get_program()
    in_maps = make_in_maps(query, keys, values, Wq, Wk, Wv, Wo)
    res = bass_utils.run_bass_kernel_spmd(nc, in_maps, core_ids=list(range(NCORES)))
    return np.concatenate(
        [res.results[i]["out"] for i in range(NCORES)], axis=0)

